# revision 1
# baseline (speedup 1.0000x reference)
"""Trainium2 Bass kernel for the CgpHmm scaled-forward log-likelihood.

Strategy (hardcoded for inputs [32,10000,126], A [132,132], B [132,126], I0 [132]):
  - Each sequence's recursion is split at t=5000: a forward chain
    alpha_t = e_t * (A^T alpha_{t-1}) and a backward chain
    delta_t = e_t * (A delta_{t+1}).  Both have the identical compute shape
    z <- e * (L^T z) with L = A (fwd) or A^T (bwd), so a single SPMD program
    runs on all 8 cores: cores 0-3 forward (4 x 8 sequences), cores 4-7
    backward with the time axis pre-reversed on the host.
  - State-major layout [132 = 128+4 partitions, 8 seqs].  Four matmuls per
    step (128/4 block split of L), bf16 stationaries, fused u/v PSUM bank.
  - Emissions are computed on-device from the one-hot inputs (bf16, exact)
    via DMA-transpose + matmul against B^T — this is the memory-bound part.
  - Normalization every 16 steps: ones-matmul broadcast sum, Ln on ScalarE,
    reciprocal+rescale on VectorE.  Host combines:
    loglik = ll_f + ll_b + log(alpha^T A delta)  (exact; rescale errors cancel).
"""

import numpy as np
import ml_dtypes

bf16 = ml_dtypes.bfloat16

BATCH, T, AB = 32, 10000, 126
NS, NU, NV = 132, 128, 4
ABP = 128          # padded alphabet
B_SEQ = 8          # sequences per core
NCORE = 8
TC = 256           # time chunk (64KB DMA-transposes — at the efficiency knee)
NCH = 20           # chunks per half  (TH = 5120 >= 5000)
S_STEPS = 4999     # chain steps (plus init at tau=0)
NORM = 14          # steps between norm measurements
FIRST_NORM = 8     # first measurement
NORM_DEFER = 2     # rescale applied this many steps after measurement
NB2 = 2 * B_SEQ    # fused u|v free width
ZPS_BUFS = 2       # chain psum rotation depth
SPLIT_EMUL = False # split the chain e-multiply into u/v DVE ops

# Pre-scale for Ln: c is ~126^-16 at each norm point, far outside the ACT
# Ln LUT's accurate domain.  Ln(c * LN_SCALE) keeps the input near 1; the
# host subtracts n_norms * log(LN_SCALE) afterwards.
LN_SCALE = np.float32(126.0) ** 16


def _build_nc(steps, nch, tc):
    import concourse.bacc as bacc
    import concourse.tile as tile
    from concourse import mybir

    f32 = mybir.dt.float32
    b16 = mybir.dt.bfloat16
    Ln = mybir.ActivationFunctionType.Ln

    th = nch * tc
    nc = bacc.Bacc("TRN2", target_bir_lowering=False, debug=False,
                   num_devices=NCORE)

    x = nc.dram_tensor("x", [B_SEQ, th, ABP], b16, kind="ExternalInput")
    w11 = nc.dram_tensor("w11", [NU, NU], b16, kind="ExternalInput")
    w21 = nc.dram_tensor("w21", [NV, NU], b16, kind="ExternalInput")
    w12 = nc.dram_tensor("w12", [NU, NV], b16, kind="ExternalInput")
    w22 = nc.dram_tensor("w22", [NV, NV], b16, kind="ExternalInput")
    bt1 = nc.dram_tensor("bt1", [ABP, NU], b16, kind="ExternalInput")
    bt2 = nc.dram_tensor("bt2", [ABP, NV], b16, kind="ExternalInput")
    q = nc.dram_tensor("q", [NS, 1], f32, kind="ExternalInput")
    out = nc.dram_tensor("out", [3, NU, B_SEQ], f32, kind="ExternalOutput")

    with tile.TileContext(nc) as tcx:
        with (
            tcx.tile_pool(name="const", bufs=1) as const,
            tcx.tile_pool(name="xt", bufs=3) as xtp,
            tcx.tile_pool(name="eps", bufs=2, space="PSUM") as epsp,
            tcx.tile_pool(name="epsv", bufs=max(1, 5 - ZPS_BUFS),
                          space="PSUM") as epsvp,
            tcx.tile_pool(name="zps", bufs=ZPS_BUFS, space="PSUM") as zpsp,
            tcx.tile_pool(name="cps", bufs=1, space="PSUM") as cpsp,
            tcx.tile_pool(name="z", bufs=4) as zp,
            tcx.tile_pool(name="small", bufs=4) as smallp,
            tcx.tile_pool(name="eil", bufs=1) as eilp,
        ):
            # ---- constants into SBUF ----
            w11s = const.tile([NU, NU], b16)
            w21s = const.tile([NV, NU], b16)
            w12s = const.tile([NU, NV], b16)
            w22s = const.tile([NV, NV], b16)
            bt1s = const.tile([ABP, NU], b16)
            bt2s = const.tile([ABP, NV], b16)
            qu = const.tile([NU, 1], f32)
            qv = const.tile([NV, 1], f32)
            for dst, src in ((w11s, w11), (w21s, w21), (w12s, w12),
                             (w22s, w22), (bt1s, bt1), (bt2s, bt2)):
                nc.sync.dma_start(out=dst[:], in_=src[:])
            nc.sync.dma_start(out=qu[:], in_=q[0:NU, :])
            nc.sync.dma_start(out=qv[:], in_=q[NU:NS, :])

            ones_u = const.tile([NU, NU], b16)
            ones_v = const.tile([NV, NU], b16)
            nc.vector.memset(ones_u[:], 1.0)
            nc.vector.memset(ones_v[:], 1.0)

            ll_acc = const.tile([1, B_SEQ], f32)
            nc.vector.memset(ll_acc[:], 0.0)

            # emission buffers (ping/pong), zero once so pad regions stay 0
            e_il0 = eilp.tile([NU, tc * NB2], f32, tag="eil0")
            e_il1 = eilp.tile([NU, tc * NB2], f32, tag="eil1")
            e_il = [e_il0, e_il1]
            nc.gpsimd.memset(e_il[0][:], 0.0)
            nc.gpsimd.memset(e_il[1][:], 0.0)

            # chain psum tiles (ping/pong), zero once so pad region stays 0
            zps = []
            for _zi in range(ZPS_BUFS):
                _zt = zpsp.tile([NU, NB2], f32, tag="zps", name=f"zps{_zi}")
                nc.vector.memset(_zt[:], 0.0)
                zps.append(_zt)

            cps = cpsp.tile([NU, B_SEQ], f32)

            # ---- emission chunk production ----
            # one unit = one sequence's slice of a chunk; units are emitted
            # interleaved with chain steps so the big N=512 matmuls and ACT
            # copies never burst ahead of chain matmuls in the engine FIFOs.
            def emit_unit(ci, s):
                buf = e_il[ci % 2]
                bufv = buf.rearrange("p (t k) -> p t k", k=NB2)
                xt = xtp.tile([ABP, tc], b16, tag="xt", name=f"xt_{ci}_{s}")
                nc.sync.dma_start_transpose(
                    out=xt[:], in_=x[s, ci * tc:(ci + 1) * tc, :])
                pu = epsp.tile([NU, tc], f32, tag="eps", name=f"pu_{ci}_{s}")
                nc.tensor.matmul(pu[:], lhsT=bt1s[:], rhs=xt[:],
                                 start=True, stop=True)
                pv = epsvp.tile([NV, tc], f32, tag="epsv", name=f"pv_{ci}_{s}")
                nc.tensor.matmul(pv[:], lhsT=bt2s[:], rhs=xt[:],
                                 start=True, stop=True)
                nc.scalar.copy(out=bufv[:, :, s], in_=pu[:])
                nc.scalar.copy(out=bufv[0:NV, :, B_SEQ + s], in_=pv[:])

            def produce_chunk(ci):
                for s in range(B_SEQ):
                    emit_unit(ci, s)

            produce_chunk(0)
            pending = [(1, s) for s in range(B_SEQ)] if nch > 1 else []
            unit_every = max(1, tc // B_SEQ)

            # ---- init: z0 = q * e_0 ----
            buf0 = e_il[0]
            z0 = zp.tile([NU, NB2], b16, tag="z")
            nc.vector.tensor_scalar_mul(z0[:, 0:B_SEQ], buf0[:, 0:B_SEQ], qu[:])
            nc.vector.tensor_scalar_mul(z0[0:NV, B_SEQ:NB2],
                                        buf0[0:NV, B_SEQ:NB2], qv[:])
            zu, zv = z0[:, 0:B_SEQ], z0[0:NV, B_SEQ:NB2]

            # ---- main chain ----
            for tau in range(1, steps + 1):
                ci, t = divmod(tau, tc)
                if t == 0 and ci + 1 < nch:
                    pending.extend((ci + 1, s) for s in range(B_SEQ))
                if pending and tau % unit_every == unit_every // 2:
                    emit_unit(*pending.pop(0))
                buf = e_il[ci % 2]
                zpt = zps[tau % ZPS_BUFS]
                nc.tensor.matmul(zpt[:, 0:B_SEQ], lhsT=w11s[:], rhs=zu,
                                 start=True, stop=False)
                nc.tensor.matmul(zpt[:, 0:B_SEQ], lhsT=w21s[:], rhs=zv,
                                 start=False, stop=True)
                nc.tensor.matmul(zpt[0:NV, B_SEQ:NB2], lhsT=w12s[:], rhs=zu,
                                 start=True, stop=False)
                nc.tensor.matmul(zpt[0:NV, B_SEQ:NB2], lhsT=w22s[:], rhs=zv,
                                 start=False, stop=True)
                z = zp.tile([NU, NB2], b16, tag="z")
                if SPLIT_EMUL:
                    nc.vector.tensor_mul(z[:, 0:B_SEQ], zpt[:, 0:B_SEQ],
                                         buf[:, t * NB2:t * NB2 + B_SEQ])
                    nc.vector.tensor_mul(z[0:NV, B_SEQ:NB2],
                                         zpt[0:NV, B_SEQ:NB2],
                                         buf[0:NV, t * NB2 + B_SEQ:(t + 1) * NB2])
                else:
                    nc.vector.tensor_mul(z[:], zpt[:],
                                         buf[:, t * NB2:(t + 1) * NB2])
                zu, zv = z[:, 0:B_SEQ], z[0:NV, B_SEQ:NB2]

                # measurement: sum z, log to ll, reciprocal -> rc (deferred)
                if (tau - FIRST_NORM) % NORM == 0 and \
                        FIRST_NORM <= tau <= steps - NORM_DEFER:
                    nc.tensor.matmul(cps[:], lhsT=ones_u[:], rhs=zu,
                                     start=True, stop=False)
                    nc.tensor.matmul(cps[:], lhsT=ones_v[:], rhs=zv,
                                     start=False, stop=True)
                    llt = smallp.tile([1, B_SEQ], f32, tag="llt")
                    nc.scalar.activation(llt[:], cps[0:1, :], Ln,
                                         scale=float(LN_SCALE))
                    nc.vector.tensor_add(ll_acc[:], ll_acc[:], llt[:])
                    rc = smallp.tile([NU, B_SEQ], b16, tag="rc")
                    # rc precision is self-correcting: the rescale that z
                    # actually receives is folded back in via the final
                    # host-side dot product; only log(c) enters ll.
                    with nc.allow_low_precision(reason="self-correcting rescale"):
                        nc.vector.reciprocal(rc[:], cps[:])
                    pending_rc = rc
                # application: two steps later, rc is ready -> cheap rescale
                if (tau - FIRST_NORM - NORM_DEFER) % NORM == 0 and \
                        tau >= FIRST_NORM + NORM_DEFER:
                    zn = zp.tile([NU, NB2], b16, tag="z")
                    nc.vector.tensor_mul(zn[:, 0:B_SEQ], zu, pending_rc[:])
                    nc.vector.tensor_mul(zn[0:NV, B_SEQ:NB2], zv,
                                         pending_rc[0:NV, :])
                    zu, zv = zn[:, 0:B_SEQ], zn[0:NV, B_SEQ:NB2]

            # ---- outputs ----
            zout = smallp.tile([NU, NB2], f32, tag="zout")
            nc.vector.tensor_copy(zout[:, 0:B_SEQ], zu)
            nc.vector.tensor_copy(zout[0:NV, B_SEQ:NB2], zv)
            nc.sync.dma_start(out=out[0, :, :], in_=zout[:, 0:B_SEQ])
            nc.sync.dma_start(out=out[1, 0:NV, :], in_=zout[0:NV, B_SEQ:NB2])
            nc.sync.dma_start(out=out[2, 0:1, :], in_=ll_acc[:])

    nc.compile()
    return nc


def _host_prep(inputs, A, B, I0, steps=S_STEPS, nch=NCH, tc=TC):
    """Build the 8 per-core input maps."""
    th = nch * tc
    half = steps + 1
    X = np.ascontiguousarray(inputs).astype(bf16)  # exact for one-hot

    A32 = np.asarray(A, np.float32)
    B32 = np.asarray(B, np.float32)
    I032 = np.asarray(I0, np.float32)

    Bt = np.zeros((ABP, NS), np.float32)
    Bt[:AB, :] = B32.T
    bt1 = Bt[:, 0:NU].astype(bf16)
    bt2 = Bt[:, NU:NS].astype(bf16)

    def wtiles(L):
        L = L.astype(bf16)
        return {
            "w11": np.ascontiguousarray(L[0:NU, 0:NU]),
            "w21": np.ascontiguousarray(L[NU:NS, 0:NU]),
            "w12": np.ascontiguousarray(L[0:NU, NU:NS]),
            "w22": np.ascontiguousarray(L[NU:NS, NU:NS]),
        }

    wf = wtiles(A32)        # fwd: lhsT = A
    wb = wtiles(A32.T)      # bwd: lhsT = A^T
    qf = I032.reshape(NS, 1)
    qb = np.ones((NS, 1), np.float32)

    in_maps = []
    for c in range(NCORE):
        fwd = c < 4
        g = c % 4
        seqs = slice(g * B_SEQ, (g + 1) * B_SEQ)
        xs = np.zeros((B_SEQ, th, ABP), bf16)
        if fwd:
            xs[:, :half, :AB] = X[seqs, :half, :]
        else:
            # time-reversed: rows t = T-1 .. T-half
            xs[:, :half, :AB] = X[seqs, : T - half - 1: -1, :]
        m = {"x": xs, "bt1": bt1, "bt2": bt2,
             "q": qf if fwd else qb}
        m.update(wf if fwd else wb)
        in_maps.append(m)
    return in_maps


def _host_combine(results, A, steps=S_STEPS):
    A64 = np.asarray(A, np.float64)
    n_norms = sum(1 for tau in range(1, steps + 1)
                  if (tau - FIRST_NORM) % NORM == 0
                  and FIRST_NORM <= tau <= steps - NORM_DEFER)
    ln_corr = n_norms * np.log(np.float64(LN_SCALE))
    loglik = np.zeros(BATCH, np.float32)
    for g in range(4):
        of = results[g]["out"]
        ob = results[g + 4]["out"]
        for k in range(B_SEQ):
            alpha = np.concatenate([of[0, :, k], of[1, 0:NV, k]]).astype(np.float64)
            delta = np.concatenate([ob[0, :, k], ob[1, 0:NV, k]]).astype(np.float64)
            dot = alpha @ A64 @ delta
            loglik[g * B_SEQ + k] = (np.float64(of[2, 0, k]) - ln_corr +
                                     np.float64(ob[2, 0, k]) - ln_corr +
                                     np.log(dot))
    return loglik


_NC_CACHE = {}


def _get_nc(steps=S_STEPS, nch=NCH, tc=TC):
    key = (steps, nch, tc)
    if key not in _NC_CACHE:
        _NC_CACHE[key] = _build_nc(steps, nch, tc)
    return _NC_CACHE[key]


def kernel(inputs, A, B, I0, trace=False):
    from concourse.bass_utils import run_bass_kernel_spmd

    nc = _get_nc()
    in_maps = _host_prep(inputs, A, B, I0)
    res = run_bass_kernel_spmd(nc, in_maps, list(range(NCORE)), trace=trace)
    out = _host_combine(res.results, A)
    if trace:
        return out, res
    return out



# revision 6
# speedup vs baseline: 2.6931x; 2.6931x over previous
"""Trainium2 Bass kernel for the CgpHmm scaled-forward log-likelihood.

Strategy (hardcoded for inputs [32,10000,126], A [132,132], B [132,126], I0 [132]):
  The HMM forward recursion contracts exponentially fast (dense positive
  softmax A mixes in ~5 steps), so each sequence is split into S=128
  segments of 79 owned steps plus a W=16 step burn-in from a uniform
  vector.  Chain length per lane: CL=95 serial steps (vs 5000 for a
  fwd/bwd split) -- throughput-bound instead of latency-bound.
  Burn-in truncation error measured at ~1e-11 relative (f64), total
  pipeline error ~6e-6 with bf16 emissions / bf16 z (tolerance 2e-2).

  Layout: 8 cores x 4 groups x 128 lanes; group g of core c runs all 128
  segments of sequence 4c+g as the matmul free dim.  State-major
  [132 = 128(u) + 4(v)] partitions, 4 matmuls/step (128/4 block split of
  lhsT=A), fused u|v PSUM tile [128, 256], two compact DVE e-multiplies.

  Emissions e'_t = 126*B.T[obs_t] are gathered on the host (bf16) in the
  exact SBUF layout and streamed in triple-buffered chunks; the 126
  pre-scaling keeps z ~O(1) so only 2 mid-chain rescales are needed.
  Lane j=0 of each sequence reproduces the exact alpha_0 = I0*e_0 initial
  condition via host-crafted burn-in emissions (ones for 14 steps, then
  e_crafted = I0*e'_0 / (A^T (A^T)^{W-1} u) at tau=W-1).  Segment sums
  (3 per lane) are the only output; all logs happen on the host in f64:
    ll_seg = log m2 + log m1 (+ log m0 for j=0),  ll -= T*log(126).
  Pad steps use e=1 which preserves the sum exactly (A row-stochastic),
  so overhang lanes contribute exactly 0.
"""

import numpy as np
import ml_dtypes

bf16 = ml_dtypes.bfloat16

BATCH, T, AB = 32, 10000, 126
NS, NU, NV = 132, 128, 4
NCORE = 8
S = 128           # segments per sequence
W = 16            # burn-in steps (incl. boundary measurement step)
OWNED = 79        # owned steps per segment (79*128 = 10112 >= 10000)
CL = OWNED + W    # 95 chain steps per lane
G = 4             # lane groups per core (one sequence per group)
LAN = 128         # lanes (segments) per group
CS = 19           # emission chunk size in steps (CL = 5*19)
NCHUNK = CL // CS
M_TAUS = (W - 1, 53, CL - 1)   # sum measurements; first two get rescales
RESCALE_DEFER = 2

LOG126 = np.log(np.float64(126.0))


def _build_nc():
    import concourse.bacc as bacc
    import concourse.tile as tile
    from concourse import mybir

    f32 = mybir.dt.float32
    b16 = mybir.dt.bfloat16

    nc = bacc.Bacc("TRN2", target_bir_lowering=False, debug=False,
                   num_devices=NCORE)

    eu = nc.dram_tensor("eu", [G, NU, CL * LAN], b16, kind="ExternalInput")
    ev = nc.dram_tensor("ev", [G, NV, CL * LAN], b16, kind="ExternalInput")
    w11 = nc.dram_tensor("w11", [NU, NU], b16, kind="ExternalInput")
    w21 = nc.dram_tensor("w21", [NV, NU], b16, kind="ExternalInput")
    w12 = nc.dram_tensor("w12", [NU, NV], b16, kind="ExternalInput")
    w22 = nc.dram_tensor("w22", [NV, NV], b16, kind="ExternalInput")
    out = nc.dram_tensor("out", [G, 1, 3 * LAN], f32, kind="ExternalOutput")

    with tile.TileContext(nc) as tcx:
        with (
            tcx.tile_pool(name="const", bufs=1) as const,
            tcx.tile_pool(name="eu", bufs=3 * G) as eup,
            tcx.tile_pool(name="ev", bufs=3 * G) as evp,
            tcx.tile_pool(name="zps", bufs=6, space="PSUM") as zpsp,
            tcx.tile_pool(name="cps", bufs=2, space="PSUM") as cpsp,
            tcx.tile_pool(name="z", bufs=3 * G) as zp,
            tcx.tile_pool(name="small", bufs=2 * G) as smallp,
        ):
            # ---- constants ----
            w11s = const.tile([NU, NU], b16)
            w21s = const.tile([NV, NU], b16)
            w12s = const.tile([NU, NV], b16)
            w22s = const.tile([NV, NV], b16)
            for dst, src in ((w11s, w11), (w21s, w21), (w12s, w12),
                             (w22s, w22)):
                nc.sync.dma_start(out=dst[:], in_=src[:])
            ones_u = const.tile([NU, NU], b16)
            ones_v = const.tile([NV, NU], b16)
            nc.vector.memset(ones_u[:], 1.0)
            nc.vector.memset(ones_v[:], 1.0)

            z0 = const.tile([NU, 2 * LAN], b16)
            nc.vector.memset(z0[:], 1.0 / NS)

            meas = [const.tile([1, 3 * LAN], f32, name=f"meas{g}")
                    for g in range(G)]

            # ---- emission chunk DMA (triple buffered per group) ----
            eut = {}
            evt = {}

            def load_chunk(g, c):
                et = eup.tile([NU, CS * LAN], b16, tag="eu",
                              name=f"eu_{g}_{c}")
                vt = evp.tile([NV, CS * LAN], b16, tag="ev",
                              name=f"ev_{g}_{c}")
                sl = slice(c * CS * LAN, (c + 1) * CS * LAN)
                nc.sync.dma_start(out=et[:], in_=eu[g, :, sl])
                nc.sync.dma_start(out=vt[:], in_=ev[g, :, sl])
                eut[(g, c)] = et
                evt[(g, c)] = vt

            for c in range(min(3, NCHUNK)):
                for g in range(G):
                    load_chunk(g, c)

            zu = [z0[:, 0:LAN] for _ in range(G)]
            zv = [z0[0:NV, LAN:2 * LAN] for _ in range(G)]
            rc_pend = [None] * G
            mi = 0  # measurement index

            for tau in range(CL):
                c, off = divmod(tau, CS)
                off *= LAN
                if off == 0 and c + 3 <= NCHUNK - 1:
                    for g in range(G):
                        load_chunk(g, c + 3)
                for g in range(G):
                    zpt = zpsp.tile([NU, 2 * LAN], f32, tag="zps",
                                    name=f"zps{g}_{tau}")
                    nc.tensor.matmul(zpt[:, 0:LAN], lhsT=w21s[:], rhs=zv[g],
                                     start=True, stop=False)
                    nc.tensor.matmul(zpt[:, 0:LAN], lhsT=w11s[:], rhs=zu[g],
                                     start=False, stop=True)
                    nc.tensor.matmul(zpt[0:NV, LAN:2 * LAN], lhsT=w12s[:],
                                     rhs=zu[g], start=True, stop=False)
                    nc.tensor.matmul(zpt[0:NV, LAN:2 * LAN], lhsT=w22s[:],
                                     rhs=zv[g], start=False, stop=True)
                    z = zp.tile([NU, 2 * LAN], b16, tag="z",
                                name=f"z{g}_{tau}")
                    e_u = eut[(g, c)]
                    e_v = evt[(g, c)]
                    nc.vector.tensor_mul(z[:, 0:LAN], zpt[:, 0:LAN],
                                         e_u[:, off:off + LAN])
                    nc.vector.tensor_mul(z[0:NV, LAN:2 * LAN],
                                         zpt[0:NV, LAN:2 * LAN],
                                         e_v[:, off:off + LAN])
                    zu[g] = z[:, 0:LAN]
                    zv[g] = z[0:NV, LAN:2 * LAN]

                    # deferred rescale from the measurement 2 steps ago
                    if tau - RESCALE_DEFER in M_TAUS[:2]:
                        zn = zp.tile([NU, 2 * LAN], b16, tag="z",
                                     name=f"zn{g}_{tau}")
                        nc.vector.tensor_mul(zn[:, 0:LAN], zu[g],
                                             rc_pend[g][:])
                        nc.vector.tensor_mul(zn[0:NV, LAN:2 * LAN], zv[g],
                                             rc_pend[g][0:NV, :])
                        zu[g] = zn[:, 0:LAN]
                        zv[g] = zn[0:NV, LAN:2 * LAN]

                    # sum measurement (broadcast across partitions via ones)
                    if tau in M_TAUS:
                        cps = cpsp.tile([NU, LAN], f32, tag="cps",
                                        name=f"cps{g}_{tau}")
                        nc.tensor.matmul(cps[:], lhsT=ones_u[:], rhs=zu[g],
                                         start=True, stop=False)
                        nc.tensor.matmul(cps[:], lhsT=ones_v[:], rhs=zv[g],
                                         start=False, stop=True)
                        nc.scalar.copy(
                            out=meas[g][0:1, mi * LAN:(mi + 1) * LAN],
                            in_=cps[0:1, :])
                        if tau != M_TAUS[2]:
                            rc = smallp.tile([NU, LAN], b16, tag="rc",
                                             name=f"rc{g}_{tau}")
                            # rescale precision self-corrects via host logs
                            with nc.allow_low_precision(
                                    reason="self-correcting rescale"):
                                nc.vector.reciprocal(rc[:], cps[:])
                            rc_pend[g] = rc
                if tau in M_TAUS:
                    mi += 1

            for g in range(G):
                nc.sync.dma_start(out=out[g, :, :], in_=meas[g][:])

    nc.compile()
    return nc


def _host_prep(inputs, A, B, I0):
    """Build the 8 per-core input maps (emissions in exact SBUF layout)."""
    A64 = np.asarray(A, np.float64)
    B64 = np.asarray(B, np.float64)
    I064 = np.asarray(I0, np.float64)
    X = np.asarray(inputs, np.float32)

    # obs via exact dot with arange (one-hot inputs, values < 126 exact f32)
    obs = X.reshape(-1, AB).dot(np.arange(AB, dtype=np.float32))
    obs = obs.reshape(BATCH, T).astype(np.int32)

    Etab64 = 126.0 * B64.T                     # [126, 132]
    Etab = Etab64.astype(bf16)

    wtiles = {
        "w11": np.ascontiguousarray(A64[0:NU, 0:NU]).astype(bf16),
        "w21": np.ascontiguousarray(A64[NU:NS, 0:NU]).astype(bf16),
        "w12": np.ascontiguousarray(A64[0:NU, NU:NS]).astype(bf16),
        "w22": np.ascontiguousarray(A64[NU:NS, NU:NS]).astype(bf16),
    }

    # predicted burn-in state for the crafted lane-0 emission
    v = np.full(NS, 1.0 / NS)
    for _ in range(W - 1):
        v = A64.T @ v
    Av = A64.T @ v                              # [132]

    # time index per (segment, tau); segment 0 handled specially
    tidx = (OWNED * np.arange(S)[:, None] - W
            + np.arange(CL)[None, :])           # [S, CL]
    valid = (tidx >= 0) & (tidx < T)
    tclip = np.clip(tidx, 0, T - 1)

    in_maps = []
    for c in range(NCORE):
        eu = np.empty((G, NU, CL * LAN), bf16)
        ev = np.empty((G, NV, CL * LAN), bf16)
        for g in range(G):
            b = 4 * c + g
            E = Etab[obs[b, tclip]]             # [S, CL, 132] bf16
            E[~valid] = bf16(1.0)
            # lane 0: fake ones, crafted tau=W-1 (exact I0*e'_0), shift
            E[0, :W - 1] = bf16(1.0)
            crafted = (I064 * Etab64[obs[b, 0]]) / Av
            E[0, W - 1] = crafted.astype(bf16)
            E[0, W:CL - 1] = Etab[obs[b, 1:OWNED]]
            E[0, CL - 1] = bf16(1.0)
            Et = E.transpose(2, 1, 0).reshape(NS, CL * LAN)
            eu[g] = Et[0:NU]
            ev[g] = Et[NU:NS]
        m = {"eu": eu, "ev": ev}
        m.update(wtiles)
        in_maps.append(m)
    return in_maps


def _host_combine(results, A=None):
    loglik = np.zeros(BATCH, np.float32)
    for c in range(NCORE):
        o = np.asarray(results[c]["out"], np.float64).reshape(G, 3 * LAN)
        for g in range(G):
            m0 = o[g, 0:LAN]
            m1 = o[g, LAN:2 * LAN]
            m2 = o[g, 2 * LAN:3 * LAN]
            ll = np.log(m2).sum() + np.log(m1).sum() + np.log(m0[0])
            loglik[4 * c + g] = ll - T * LOG126
    return loglik


_NC_CACHE = {}


def _get_nc():
    if "nc" not in _NC_CACHE:
        _NC_CACHE["nc"] = _build_nc()
    return _NC_CACHE["nc"]


def kernel(inputs, A, B, I0, trace=False):
    from concourse.bass_utils import run_bass_kernel_spmd

    nc = _get_nc()
    in_maps = _host_prep(inputs, A, B, I0)
    res = run_bass_kernel_spmd(nc, in_maps, list(range(NCORE)), trace=trace)
    out = _host_combine(res.results)
    if trace:
        return out, res
    return out
